# revision 1
# baseline (speedup 1.0000x reference)
"""Tensor-parallel attention kernel for trn2 (8 cores).

TP over heads (2/core) for QKV + attention; per-head AllToAll reshards
attention output to seq-parallel; output projection seq-sharded (each
core owns 256 output rows); host concatenates. Matmuls in float32r.

Softmax: no max-subtraction (scores ~N(0,2), exp safe in fp32).
Per kv-tile, two 512-wide q-chunks are processed against one weight
load: chunk A's mask is added on DVE, chunk B's on the PE via an
identity-matmul accumulate; chunk A's softmax denominator accumulates
on the PE (ones-vector matmul), chunk B's on DVE. 1/sums via fast
approx reciprocal on DVE.
"""
import math
import numpy as np

import concourse.bass as bass
import concourse.mybir as mybir
import concourse.tile as tile
from concourse import bacc
from concourse.masks import make_identity

f32 = mybir.dt.float32
f32r = mybir.dt.float32r
f16 = mybir.dt.float16

P = 128
S = 2048
D = 2048
HD = 128
NH = 2          # heads per core
W = 8           # cores
QS = S // W     # 256 output rows per core
DT = D // P     # 16 contraction tiles
KT = S // P     # 16 kv tiles

Exp = mybir.ActivationFunctionType.Exp
ADD = mybir.AluOpType.add
MULT = mybir.AluOpType.mult


def build():
    nc = bacc.Bacc("TRN2", target_bir_lowering=False, debug=False, num_devices=W)

    xt = nc.dram_tensor("xt", [D, S], f32r, kind="ExternalInput").ap()
    # weights pre-arranged host-side to [P, DT, NH*HD], contiguous
    wq_t = nc.dram_tensor("wq_t", [NH, P, DT, HD], f32r, kind="ExternalInput").ap()
    wk_t = nc.dram_tensor("wk_t", [NH, P, DT, HD], f32r, kind="ExternalInput").ap()
    wv_t = nc.dram_tensor("wv_t", [NH, P, DT, HD], f32r, kind="ExternalInput").ap()
    mask_t = nc.dram_tensor("mask_t", [NH, S, S], f16, kind="ExternalInput").ap()
    wo_t = nc.dram_tensor("wo_t", [D, D], f16, kind="ExternalInput").ap()
    y = nc.dram_tensor("y", [QS, D], f32, kind="ExternalOutput").ap()

    wqkv = {"q": wq_t, "k": wk_t, "v": wv_t}

    with tile.TileContext(nc) as tc:
        persist = tc.alloc_tile_pool(name="persist", bufs=1)
        consts = tc.alloc_tile_pool(name="consts", bufs=1)
        probsp = tc.alloc_tile_pool(name="probsp", bufs=8)
        smallp = tc.alloc_tile_pool(name="smallp", bufs=2)
        dram = tc.alloc_tile_pool(name="dram", bufs=1, space="DRAM")

        ident_f = consts.tile([P, P], f32, name="ident_f")
        make_identity(nc, ident_f[:])
        ident = consts.tile([P, P], f32r, name="ident")
        nc.vector.tensor_copy(out=ident[:], in_=ident_f[:])
        ident_h = consts.tile([P, P], f16, name="ident_h")
        nc.vector.tensor_copy(out=ident_h[:], in_=ident_f[:])
        ones_f = consts.tile([P, 1], f32, name="ones_f")
        nc.gpsimd.memset(ones_f[:], 1.0)
        ones_col = consts.tile([P, 1], f32r, name="ones_col")
        nc.vector.tensor_copy(out=ones_col[:], in_=ones_f[:])
        ones_rf = consts.tile([1, P], f32, name="ones_rf")
        nc.gpsimd.memset(ones_rf[:], 1.0)
        ones_row = consts.tile([1, P], f32r, name="ones_row")
        nc.vector.tensor_copy(out=ones_row[:], in_=ones_rf[:])

        qT = [persist.tile([P, S], f32r, name=f"qT{b}") for b in range(NH)]
        kT = [persist.tile([P, S], f32r, name=f"kT{b}") for b in range(NH)]
        vT = [persist.tile([P, S], f32r, name=f"vT{b}") for b in range(NH)]

        a2a_in = [dram.tile([W, P, QS], f16, name=f"a2a_in{b}") for b in range(NH)]
        a2a_out = [dram.tile([W, P, QS], f16, name=f"a2a_out{b}") for b in range(NH)]

        # ---------------- Phase A: projections ----------------
        psA = tc.alloc_tile_pool(name="psA", bufs=4, space="PSUM")
        wpool = tc.alloc_tile_pool(name="wpool", bufs=6)
        xpool = tc.alloc_tile_pool(name="xpool", bufs=1)
        HALF = S // 2
        dests = {"q": qT, "k": kT, "v": vT}
        dma_engines = [nc.sync, nc.scalar]

        with nc.named_scope("A"):
            wtiles = {}
            for kind in ("k", "v", "q"):
                for b in range(NH):
                    wt = wpool.tile([P, DT, HD], f32r, name=f"w_{kind}{b}",
                                    tag="wt")
                    nc.gpsimd.dma_start(wt[:], wqkv[kind][b])
                    wtiles[(kind, b)] = wt
            for half in range(2):
                col0 = half * HALF
                xts = []
                for t in range(DT):
                    xtile = xpool.tile([P, HALF], f32r, name=f"xt{t}",
                                       tag=f"xt{t}")
                    dma_engines[t % 2].dma_start(
                        xtile[:], xt[t * P:(t + 1) * P, col0:col0 + HALF])
                    xts.append(xtile)
                for kind in ("k", "v", "q"):
                    for b in range(NH):
                        wt = wtiles[(kind, b)]
                        dst = dests[kind][b]
                        ps0 = psA.tile([P, 512], f32, name="psa0", tag="psa")
                        ps1 = psA.tile([P, 512], f32, name="psa1", tag="psa")
                        for t in range(DT):
                            nc.tensor.matmul(ps0[:], wt[:, t, :],
                                             xts[t][:, 0:512],
                                             start=(t == 0), stop=(t == DT - 1))
                            nc.tensor.matmul(ps1[:], wt[:, t, :],
                                             xts[t][:, 512:1024],
                                             start=(t == 0), stop=(t == DT - 1))
                        nc.scalar.copy(dst[:, col0:col0 + 512], ps0[:])
                        nc.scalar.copy(dst[:, col0 + 512:col0 + 1024], ps1[:])
        xpool.release()
        wpool.release()
        psA.release()

        # ---------------- Phase B: attention ----------------
        maskp = tc.alloc_tile_pool(name="maskp", bufs=1)
        vpool = tc.alloc_tile_pool(name="vpool", bufs=1)
        psS = tc.alloc_tile_pool(name="psS", bufs=1, space="PSUM")
        psPV = tc.alloc_tile_pool(name="psPV", bufs=1, space="PSUM")

        v_sb = [vpool.tile([P, KT, P], f32r, name=f"v_sb{b}") for b in range(NH)]
        # v transpose: vT [dh, S] -> v_sb [kv, kv_tile, dh]
        for b in range(NH):
          with nc.named_scope("T"):
            for t in range(KT):
                pst = psS.tile([P, P], f32r, name="pst", tag="small", bufs=1)
                nc.tensor.transpose(pst[:], vT[b][:, t * P:(t + 1) * P], ident[:])
                nc.vector.tensor_copy(out=v_sb[b][:, t, :], in_=pst[:])

        for b in range(NH):
          with nc.named_scope(f"B{b}"):
            for pair in range(2):
                base = pair * 1024
                slA = slice(base, base + 512)
                slB = slice(base + 512, base + 1024)
                mts = []
                for t in range(KT):
                    mtile = maskp.tile([P, 1024], f16, name=f"mt{t}",
                                       tag=f"mt{t}")
                    nc.gpsimd.dma_start(
                        mtile[:],
                        mask_t[b, t * P:(t + 1) * P, base:base + 1024])
                    mts.append(mtile)
                pv0 = psPV.tile([P, 512], f32, name="pv0", tag="pv0")
                pv1 = psPV.tile([P, 512], f32, name="pv1", tag="pv1")
                sums_ps = psS.tile([1, 512], f32, name="sums_ps",
                                   tag="sums_ps", bufs=1)
                sacc = smallp.tile([P, 512], f32r, name="sacc", tag="sacc")
                PIPE = 2
                pq = {}
                for tt in range(KT + PIPE):
                    if tt < KT:
                        t = tt
                        ktile = kT[b][:, t * P:(t + 1) * P]
                        ps_sA = psS.tile([P, 512], f32, name="ps_sA",
                                         tag="ps_s", bufs=4)
                        nc.tensor.matmul(ps_sA[:], ktile, qT[b][:, slA],
                                         start=True, stop=True)
                        ps_sB = psS.tile([P, 512], f32, name="ps_sB",
                                         tag="ps_s", bufs=4)
                        nc.tensor.matmul(ps_sB[:], ktile, qT[b][:, slB],
                                         start=True, stop=False)
                        nc.tensor.matmul(ps_sB[:], ident_h[:],
                                         mts[t][:, 512:1024],
                                         start=False, stop=True)
                        prA = probsp.tile([P, 512], f32r, name="prA", tag="probs")
                        nc.vector.tensor_tensor(out=prA[:], in0=ps_sA[:],
                                                in1=mts[t][:, 0:512], op=ADD)
                        nc.scalar.activation(prA[:], prA[:], Exp)
                        prB = probsp.tile([P, 512], f32r, name="prB", tag="probs")
                        nc.scalar.activation(prB[:], ps_sB[:], Exp)
                        pq[t] = (prA, prB)
                    if tt >= PIPE:
                        t = tt - PIPE
                        prA, prB = pq.pop(t)
                        vtile = v_sb[b][:, t, :]
                        nc.tensor.matmul(pv0[:], vtile, prA[:],
                                         start=(t == 0), stop=(t == KT - 1))
                        nc.tensor.matmul(pv1[:], vtile, prB[:],
                                         start=(t == 0), stop=(t == KT - 1))
                        nc.tensor.matmul(sums_ps[:], ones_col[:], prA[:],
                                         start=(t == 0), stop=(t == KT - 1))
                        if t == 0:
                            nc.vector.tensor_copy(out=sacc[:], in_=prB[:])
                        else:
                            nc.vector.tensor_tensor(out=sacc[:], in0=sacc[:],
                                                    in1=prB[:], op=ADD)
                # epilogues: chunk A (sums in psum), then chunk B (sacc)
                for chunk, pv in ((0, pv0), (1, pv1)):
                    qc = 2 * pair + chunk
                    if chunk == 1:
                        nc.tensor.matmul(sums_ps[:], ones_col[:], sacc[:],
                                         start=True, stop=True)
                    sums_row = smallp.tile([1, 512], f32r, name="sums_row",
                                           tag="sums_row")
                    nc.scalar.copy(sums_row[:], sums_ps[:])
                    ps_bc = psS.tile([P, 512], f32, name="ps_bc", tag="small",
                                     bufs=1)
                    nc.tensor.matmul(ps_bc[:], ones_row[:], sums_row[:],
                                     start=True, stop=True)
                    bc_sb = smallp.tile([P, 512], f32, name="bc_sb", tag="bc_sb")
                    nc.vector.reciprocal_approx_fast(out=bc_sb[:], in_=ps_bc[:])
                    attn_sb = smallp.tile([P, 512], f16, name="attn_sb",
                                          tag="attn_sb")
                    nc.vector.tensor_tensor(out=attn_sb[:], in0=pv[:],
                                            in1=bc_sb[:], op=MULT)
                    for jj in range(2):
                        j = qc * 2 + jj
                        nc.sync.dma_start(a2a_in[b][j, :, :],
                                          attn_sb[:, jj * QS:(jj + 1) * QS])
            nc.gpsimd.collective_compute(
                "AllToAll", mybir.AluOpType.bypass,
                replica_groups=[list(range(W))],
                ins=[a2a_in[b].opt()],
                outs=[a2a_out[b].opt()],
            )
        psPV.release()
        psS.release()

        # ---------------- Phase D: output projection ----------------
        psD = tc.alloc_tile_pool(name="psD", bufs=1, space="PSUM")
        dpool = tc.alloc_tile_pool(name="dpool", bufs=3)
        ps_y = [[psD.tile([P, 512], f32, name=f"ps_y{qb}_{dc}", tag=f"ps_y{qb}_{dc}")
                 for dc in range(4)] for qb in range(2)]
        first = True
        for b in range(NH):
            for i in range(W):
                at = dpool.tile([P, QS], f16, name="at", tag="at", bufs=5)
                nc.sync.dma_start(at[:], a2a_out[b][i, :, :])
                h = 2 * i + b
                wo_sb = dpool.tile([P, D], f16, name="wo_sb", tag="wo_sb", bufs=8)
                dma_engines[i % 2].dma_start(wo_sb[:], wo_t[h * P:(h + 1) * P, :])
                last = (b == NH - 1) and (i == W - 1)
                for qb in range(2):
                    for dc in range(4):
                        nc.tensor.matmul(ps_y[qb][dc][:],
                                         at[:, qb * P:(qb + 1) * P],
                                         wo_sb[:, dc * 512:(dc + 1) * 512],
                                         start=first, stop=last)
                first = False
        for qb in range(2):
            y_sb = dpool.tile([P, D], f32, name="y_sb", tag="y_sb", bufs=1)
            for dc in range(4):
                nc.vector.tensor_copy(out=y_sb[:, dc * 512:(dc + 1) * 512],
                                      in_=ps_y[qb][dc][:])
            nc.sync.dma_start(y[qb * P:(qb + 1) * P, :], y_sb[:])
        dpool.release()
        psD.release()

        for p in [vpool, maskp, dram, smallp, probsp, consts, persist]:
            p.release()

    nc.compile()
    return nc


def _warr(w, scale=None):
    """[D, 256] -> [NH, P, DT, HD] host layout (contiguous per head)."""
    wt = w.T if scale is None else (w * scale).T          # [D, 256]
    a = wt.reshape(DT, P, NH, HD).transpose(2, 1, 0, 3)   # [NH, P, DT, HD]
    return np.ascontiguousarray(a)


def make_in_maps(x, mask, wq, wk, wv, wo):
    """x [1,S,D]; mask [1,16,S,S]; w* [D,D] (all f32) -> per-core dicts."""
    scale = np.float32(1.0 / math.sqrt(HD))
    xtv = np.ascontiguousarray(x[0].T)
    wo_tv = np.ascontiguousarray(wo.T).astype(np.float16)
    in_maps = []
    for c in range(W):
        rows = slice(NH * HD * c, NH * HD * (c + 1))
        m = mask[0, NH * c:NH * (c + 1)]
        in_maps.append({
            "xt": xtv,
            "wq_t": _warr(wq[rows], scale),
            "wk_t": _warr(wk[rows]),
            "wv_t": _warr(wv[rows]),
            "mask_t": np.ascontiguousarray(m.transpose(0, 2, 1)).astype(np.float16),
            "wo_t": wo_tv,
        })
    return in_maps


def assemble(results):
    return np.concatenate([results[c]["y"] for c in range(W)], axis=0)[None]


# ----------------------------------------------------------------------
# Harness entry point: kernel(**inputs) takes the FULL unsharded inputs
# as produced by setup_inputs() and returns the FULL [1, S, D] output.
# Inside: inputs are sharded head-wise (TP) across the 8 NeuronCores,
# the Bass kernel runs SPMD (with two AllToAll collectives), and the
# seq-sharded outputs are concatenated on the host.
# ----------------------------------------------------------------------
_NC_CACHE = []


def kernel(x, mask, start_pos, wq, wk, wv, wo):
    from concourse import bass_utils
    x = np.asarray(x, dtype=np.float32)
    mask = np.asarray(mask, dtype=np.float32)
    wq = np.asarray(wq, dtype=np.float32)
    wk = np.asarray(wk, dtype=np.float32)
    wv = np.asarray(wv, dtype=np.float32)
    wo = np.asarray(wo, dtype=np.float32)
    # start_pos == 0 prefill (as in the reference)
    if not _NC_CACHE:
        _NC_CACHE.append(build())
    nc = _NC_CACHE[0]
    in_maps = make_in_maps(x, mask, wq, wk, wv, wo)
    res = bass_utils.run_bass_kernel_spmd(nc, in_maps, core_ids=list(range(W)))
    return assemble(res.results).astype(np.float32)



# revision 8
# speedup vs baseline: 1.1086x; 1.1086x over previous
"""Tensor-parallel attention kernel for trn2 (8 cores).

TP over heads (2/core) for QKV + attention; per-head AllToAll reshards
attention output to seq-parallel; output projection seq-sharded (each
core owns 256 output rows); host concatenates.

v2: all-fp16 data path (x/w/q/k/v/probs) for half DMA + fast weight
load. Mask is applied as exp(mask) (precomputed host-side) via a
multiplicative fp16 DVE op after the ACT exp, so the PE does only the
essential matmuls (QKV, QK^T, PV, partition-reduce + broadcast, WO).
Softmax sums accumulate on DVE in fp16 (2x mode). Phase A computes
K,V before Q per half so attention for head 0 starts ~30us earlier.
"""
import math
import numpy as np

import concourse.bass as bass
import concourse.mybir as mybir
import concourse.tile as tile
from concourse import bacc
from concourse.masks import make_identity

f32 = mybir.dt.float32
f32r = mybir.dt.float32r
f16 = mybir.dt.float16

P = 128
S = 2048
D = 2048
HD = 128
NH = 2          # heads per core
W = 8           # cores
QS = S // W     # 256 output rows per core
DT = D // P     # 16 contraction tiles
KT = S // P     # 16 kv tiles
QB = 1024       # q block width
NB = S // QB    # q blocks per head

Exp = mybir.ActivationFunctionType.Exp
ADD = mybir.AluOpType.add
MULT = mybir.AluOpType.mult


def build():
    nc = bacc.Bacc("TRN2", target_bir_lowering=False, debug=False, num_devices=W)

    xt = nc.dram_tensor("xt", [D, S], f16, kind="ExternalInput").ap()
    # weights pre-arranged host-side to [NH, P, DT, HD], contiguous
    wq_t = nc.dram_tensor("wq_t", [NH, P, DT, HD], f16, kind="ExternalInput").ap()
    wk_t = nc.dram_tensor("wk_t", [NH, P, DT, HD], f16, kind="ExternalInput").ap()
    wv_t = nc.dram_tensor("wv_t", [NH, P, DT, HD], f16, kind="ExternalInput").ap()
    # emask = exp(mask), transposed to [NH, S_kv, S_q]
    emask_t = nc.dram_tensor("emask_t", [NH, S, S], f16, kind="ExternalInput").ap()
    wo_t = nc.dram_tensor("wo_t", [D, D], f16, kind="ExternalInput").ap()
    y = nc.dram_tensor("y", [QS, D], f32, kind="ExternalOutput").ap()

    wqkv = {"q": wq_t, "k": wk_t, "v": wv_t}

    with tile.TileContext(nc) as tc:
        persist = tc.alloc_tile_pool(name="persist", bufs=1)
        consts = tc.alloc_tile_pool(name="consts", bufs=1)
        probsp = tc.alloc_tile_pool(name="probsp", bufs=5)
        smallp = tc.alloc_tile_pool(name="smallp", bufs=2)
        dram = tc.alloc_tile_pool(name="dram", bufs=1, space="DRAM")

        ident_f = consts.tile([P, P], f32, name="ident_f")
        make_identity(nc, ident_f[:])
        ident_h = consts.tile([P, P], f16, name="ident_h")
        nc.vector.tensor_copy(out=ident_h[:], in_=ident_f[:])
        ones_f = consts.tile([P, 1], f32, name="ones_f")
        nc.gpsimd.memset(ones_f[:], 1.0)
        ones_col = consts.tile([P, 1], f16, name="ones_col")
        nc.vector.tensor_copy(out=ones_col[:], in_=ones_f[:])
        ones_rf = consts.tile([1, P], f32, name="ones_rf")
        nc.gpsimd.memset(ones_rf[:], 1.0)
        ones_row = consts.tile([1, P], f16, name="ones_row")
        nc.vector.tensor_copy(out=ones_row[:], in_=ones_rf[:])

        qT = [persist.tile([P, S], f16, name=f"qT{b}") for b in range(NH)]
        kT = [persist.tile([P, S], f16, name=f"kT{b}") for b in range(NH)]
        vT = [persist.tile([P, S], f16, name=f"vT{b}") for b in range(NH)]

        a2a_in = [dram.tile([W, P, QS], f16, name=f"a2a_in{b}") for b in range(NH)]
        a2a_out = [dram.tile([W, P, QS], f16, name=f"a2a_out{b}") for b in range(NH)]

        # ---------------- Phase A: projections ----------------
        psA = tc.alloc_tile_pool(name="psA", bufs=2, space="PSUM")
        wpool = tc.alloc_tile_pool(name="wpool", bufs=6)
        xpool = tc.alloc_tile_pool(name="xpool", bufs=1)
        HALF = S // 2
        dests = {"q": qT, "k": kT, "v": vT}
        dma_engines = [nc.sync, nc.scalar]

        def proj(kind, b, xts, col0):
            wt = wtiles[(kind, b)]
            dst = dests[kind][b]
            for c in range(2):
                ps = psA.tile([P, 512], f32, name="psa", tag="psa")
                for t in range(DT):
                    nc.tensor.matmul(ps[:], wt[:, t, :],
                                     xts[t][:, c * 512:(c + 1) * 512],
                                     start=(t == 0), stop=(t == DT - 1))
                nc.scalar.copy(dst[:, col0 + c * 512:col0 + (c + 1) * 512],
                               ps[:])

        with nc.named_scope("A"):
            wtiles = {}
            for kind in ("k", "v", "q"):
                for b in range(NH):
                    wt = wpool.tile([P, DT, HD], f16, name=f"w_{kind}{b}",
                                    tag="wt")
                    nc.gpsimd.dma_start(wt[:], wqkv[kind][b])
                    wtiles[(kind, b)] = wt
            for half in range(2):
                col0 = half * HALF
                xts = []
                for t in range(DT):
                    xtile = xpool.tile([P, HALF], f16, name=f"x{half}_{t}",
                                       tag=f"x{half}_{t}")
                    dma_engines[t % 2].dma_start(
                        xtile[:], xt[t * P:(t + 1) * P, col0:col0 + HALF])
                    xts.append(xtile)
                if half == 0:
                    for kind in ("k", "v"):
                        for b in range(NH):
                            proj(kind, b, xts, col0)
                    for b in range(NH):
                        proj("q", b, xts, col0)
                else:
                    # head-0 K,V first so attention can begin early
                    for b in range(NH):
                        proj("k", b, xts, col0)
                        proj("v", b, xts, col0)
                    for b in range(NH):
                        proj("q", b, xts, col0)

        # ---------------- Phase B: attention ----------------
        maskp = tc.alloc_tile_pool(name="maskp", bufs=20)
        vpool = tc.alloc_tile_pool(name="vpool", bufs=1)
        psS = tc.alloc_tile_pool(name="psS", bufs=2, space="PSUM")
        psPV = tc.alloc_tile_pool(name="psPV", bufs=1, space="PSUM")

        v_sb = [vpool.tile([P, KT, P], f16, name=f"v_sb{b}") for b in range(NH)]
        # v transpose: vT [dh, S] -> v_sb [kv, kv_tile, dh]
        for b in range(NH):
          with nc.named_scope("T"):
            for t in range(KT):
                pst = psS.tile([P, P], f16, name="pst", tag="sc")
                nc.tensor.transpose(pst[:], vT[b][:, t * P:(t + 1) * P],
                                    ident_h[:])
                nc.vector.tensor_copy(out=v_sb[b][:, t, :], in_=pst[:])

        for b in range(NH):
          with nc.named_scope(f"B{b}"):
            for blk in range(NB):
                base = blk * QB
                mts = []
                for t in range(KT):
                    mtile = maskp.tile([P, QB], f16, name=f"mt{t}", tag="mt")
                    nc.gpsimd.dma_start(
                        mtile[:],
                        emask_t[b, t * P:(t + 1) * P, base:base + QB])
                    mts.append(mtile)
                pv = psPV.tile([P, QB], f32, name="pv", tag="pv")
                sacc = smallp.tile([P, QB], f16, name="sacc", tag="sacc")
                PIPE = 2
                pq = {}
                for tt in range(KT + PIPE):
                    if tt < KT:
                        t = tt
                        ktile = kT[b][:, t * P:(t + 1) * P]
                        ps_s = psS.tile([P, QB], f32, name="ps_s", tag="sc")
                        nc.tensor.matmul(ps_s[:, 0:512], ktile,
                                         qT[b][:, base:base + 512],
                                         start=True, stop=True)
                        nc.tensor.matmul(ps_s[:, 512:QB], ktile,
                                         qT[b][:, base + 512:base + QB],
                                         start=True, stop=True)
                        pr = probsp.tile([P, QB], f16, name="pr", tag="probs")
                        nc.scalar.activation(pr[:], ps_s[:], Exp)
                        nc.vector.tensor_tensor(out=pr[:], in0=pr[:],
                                                in1=mts[t][:], op=MULT)
                        if t == 0:
                            nc.vector.tensor_copy(out=sacc[:], in_=pr[:])
                        else:
                            nc.vector.tensor_tensor(out=sacc[:], in0=sacc[:],
                                                    in1=pr[:], op=ADD)
                        pq[t] = pr
                    if tt >= PIPE:
                        t = tt - PIPE
                        pr = pq.pop(t)
                        vtile = v_sb[b][:, t, :]
                        nc.tensor.matmul(pv[:, 0:512], vtile, pr[:, 0:512],
                                         start=(t == 0), stop=(t == KT - 1))
                        nc.tensor.matmul(pv[:, 512:QB], vtile, pr[:, 512:QB],
                                         start=(t == 0), stop=(t == KT - 1))
                # epilogue: partition-reduce sums, broadcast, normalize
                sums_ps = psS.tile([1, QB], f32, name="sums_ps", tag="sc")
                nc.tensor.matmul(sums_ps[:, 0:512], ones_col[:],
                                 sacc[:, 0:512], start=True, stop=True)
                nc.tensor.matmul(sums_ps[:, 512:QB], ones_col[:],
                                 sacc[:, 512:QB], start=True, stop=True)
                sums_row = smallp.tile([1, QB], f16, name="sums_row",
                                       tag="sums_row")
                nc.scalar.copy(sums_row[:], sums_ps[:])
                ps_bc = psS.tile([P, QB], f32, name="ps_bc", tag="sc")
                nc.tensor.matmul(ps_bc[:, 0:512], ones_row[:],
                                 sums_row[:, 0:512], start=True, stop=True)
                nc.tensor.matmul(ps_bc[:, 512:QB], ones_row[:],
                                 sums_row[:, 512:QB], start=True, stop=True)
                bc_sb = smallp.tile([P, QB], f32, name="bc_sb", tag="bc_sb")
                nc.vector.reciprocal_approx_fast(out=bc_sb[:], in_=ps_bc[:])
                attn_sb = smallp.tile([P, QB], f16, name="attn_sb",
                                      tag="attn_sb")
                nc.vector.tensor_tensor(out=attn_sb[:], in0=pv[:],
                                        in1=bc_sb[:], op=MULT)
                for jj in range(QB // QS):
                    j = blk * (QB // QS) + jj
                    nc.sync.dma_start(a2a_in[b][j, :, :],
                                      attn_sb[:, jj * QS:(jj + 1) * QS])
            nc.gpsimd.collective_compute(
                "AllToAll", mybir.AluOpType.bypass,
                replica_groups=[list(range(W))],
                ins=[a2a_in[b].opt()],
                outs=[a2a_out[b].opt()],
            )
        for p in [psPV, psS, vpool, maskp, xpool, wpool, psA]:
            p.release()

        # ---------------- Phase D: output projection ----------------
        psD = tc.alloc_tile_pool(name="psD", bufs=1, space="PSUM")
        dpool = tc.alloc_tile_pool(name="dpool", bufs=3)
        ps_y = [[psD.tile([P, 512], f32, name=f"ps_y{qb}_{dc}", tag=f"ps_y{qb}_{dc}")
                 for dc in range(4)] for qb in range(2)]
        first = True
        for b in range(NH):
            for i in range(W):
                at = dpool.tile([P, QS], f16, name="at", tag="at", bufs=5)
                nc.sync.dma_start(at[:], a2a_out[b][i, :, :])
                h = 2 * i + b
                wo_sb = dpool.tile([P, D], f16, name="wo_sb", tag="wo_sb", bufs=8)
                dma_engines[i % 2].dma_start(wo_sb[:], wo_t[h * P:(h + 1) * P, :])
                last = (b == NH - 1) and (i == W - 1)
                for qb in range(2):
                    for dc in range(4):
                        nc.tensor.matmul(ps_y[qb][dc][:],
                                         at[:, qb * P:(qb + 1) * P],
                                         wo_sb[:, dc * 512:(dc + 1) * 512],
                                         start=first, stop=last)
                first = False
        for qb in range(2):
            y_sb = dpool.tile([P, D], f32, name="y_sb", tag="y_sb", bufs=1)
            for dc in range(4):
                nc.vector.tensor_copy(out=y_sb[:, dc * 512:(dc + 1) * 512],
                                      in_=ps_y[qb][dc][:])
            nc.sync.dma_start(y[qb * P:(qb + 1) * P, :], y_sb[:])
        dpool.release()
        psD.release()

        for p in [dram, smallp, probsp, consts, persist]:
            p.release()

    nc.compile()
    return nc


def _warr(w, scale=None):
    """[D, 256] -> [NH, P, DT, HD] host layout (contiguous per head)."""
    wt = w.T if scale is None else (w * scale).T          # [D, 256]
    a = wt.reshape(DT, P, NH, HD).transpose(2, 1, 0, 3)   # [NH, P, DT, HD]
    return np.ascontiguousarray(a).astype(np.float16)


def make_in_maps(x, mask, wq, wk, wv, wo):
    """x [1,S,D]; mask [1,16,S,S]; w* [D,D] (all f32) -> per-core dicts."""
    scale = np.float32(1.0 / math.sqrt(HD))
    xtv = np.ascontiguousarray(x[0].T).astype(np.float16)
    wo_tv = np.ascontiguousarray(wo.T).astype(np.float16)
    in_maps = []
    for c in range(W):
        rows = slice(NH * HD * c, NH * HD * (c + 1))
        m = mask[0, NH * c:NH * (c + 1)]
        em = np.exp(m.transpose(0, 2, 1))
        in_maps.append({
            "xt": xtv,
            "wq_t": _warr(wq[rows], scale),
            "wk_t": _warr(wk[rows]),
            "wv_t": _warr(wv[rows]),
            "emask_t": np.ascontiguousarray(em).astype(np.float16),
            "wo_t": wo_tv,
        })
    return in_maps


def assemble(results):
    return np.concatenate([results[c]["y"] for c in range(W)], axis=0)[None]


# ----------------------------------------------------------------------
# Harness entry point: kernel(**inputs) takes the FULL unsharded inputs
# as produced by setup_inputs() and returns the FULL [1, S, D] output.
# Inside: inputs are sharded head-wise (TP) across the 8 NeuronCores,
# the Bass kernel runs SPMD (with two AllToAll collectives), and the
# seq-sharded outputs are concatenated on the host.
# ----------------------------------------------------------------------
_NC_CACHE = []


def kernel(x, mask, start_pos, wq, wk, wv, wo):
    from concourse import bass_utils
    x = np.asarray(x, dtype=np.float32)
    mask = np.asarray(mask, dtype=np.float32)
    wq = np.asarray(wq, dtype=np.float32)
    wk = np.asarray(wk, dtype=np.float32)
    wv = np.asarray(wv, dtype=np.float32)
    wo = np.asarray(wo, dtype=np.float32)
    # start_pos == 0 prefill (as in the reference)
    if not _NC_CACHE:
        _NC_CACHE.append(build())
    nc = _NC_CACHE[0]
    in_maps = make_in_maps(x, mask, wq, wk, wv, wo)
    res = bass_utils.run_bass_kernel_spmd(nc, in_maps, core_ids=list(range(W)))
    return assemble(res.results).astype(np.float32)


# revision 16
# speedup vs baseline: 1.1328x; 1.0218x over previous
"""Tensor-parallel attention kernel for trn2 (8 cores).

TP over heads (2/core) for QKV + attention; per-head AllToAll reshards
attention output to seq-parallel; output projection seq-sharded (each
core owns 256 output rows); host concatenates.

v2: all-fp16 data path (x/w/q/k/v/probs) for half DMA + fast weight
load. Mask is applied as exp(mask) (precomputed host-side) via a
multiplicative fp16 DVE op after the ACT exp, so the PE does only the
essential matmuls (QKV, QK^T, PV, partition-reduce + broadcast, WO).
Softmax sums accumulate on DVE in fp16 (2x mode). Phase A computes
K,V before Q per half so attention for head 0 starts ~30us earlier.
"""
import math
import numpy as np

import concourse.bass as bass
import concourse.mybir as mybir
import concourse.tile as tile
from concourse import bacc
from concourse.masks import make_identity

f32 = mybir.dt.float32
f32r = mybir.dt.float32r
f16 = mybir.dt.float16

P = 128
S = 2048
D = 2048
HD = 128
NH = 2          # heads per core
W = 8           # cores
QS = S // W     # 256 output rows per core
DT = D // P     # 16 contraction tiles
KT = S // P     # 16 kv tiles
QB = 1024       # q block width
NB = S // QB    # q blocks per head

Exp = mybir.ActivationFunctionType.Exp
ADD = mybir.AluOpType.add
MULT = mybir.AluOpType.mult


def build():
    nc = bacc.Bacc("TRN2", target_bir_lowering=False, debug=False, num_devices=W)

    xt = nc.dram_tensor("xt", [D, S], f16, kind="ExternalInput").ap()
    # weights pre-arranged host-side to [NH, P, DT, HD], contiguous
    wq_t = nc.dram_tensor("wq_t", [NH, P, DT, HD], f16, kind="ExternalInput").ap()
    wk_t = nc.dram_tensor("wk_t", [NH, P, DT, HD], f16, kind="ExternalInput").ap()
    wv_t = nc.dram_tensor("wv_t", [NH, P, DT, HD], f16, kind="ExternalInput").ap()
    # emask = exp(mask), transposed to [NH, S_kv, S_q]
    emask_t = nc.dram_tensor("emask_t", [NH, S, S], f16, kind="ExternalInput").ap()
    wo_t = nc.dram_tensor("wo_t", [D, D], f16, kind="ExternalInput").ap()
    y = nc.dram_tensor("y", [QS, D], f32, kind="ExternalOutput").ap()

    wqkv = {"q": wq_t, "k": wk_t, "v": wv_t}

    with tile.TileContext(nc) as tc:
        persist = tc.alloc_tile_pool(name="persist", bufs=1)
        consts = tc.alloc_tile_pool(name="consts", bufs=1)
        probsp = tc.alloc_tile_pool(name="probsp", bufs=5)
        smallp = tc.alloc_tile_pool(name="smallp", bufs=2)
        dram = tc.alloc_tile_pool(name="dram", bufs=1, space="DRAM")


        ident_f = consts.tile([P, P], f32, name="ident_f")
        make_identity(nc, ident_f[:])
        ident_h = consts.tile([P, P], f16, name="ident_h")
        nc.vector.tensor_copy(out=ident_h[:], in_=ident_f[:])
        allones_f = consts.tile([P, P], f32, name="allones_f")
        nc.gpsimd.memset(allones_f[:], 1.0)
        allones = consts.tile([P, P], f16, name="allones")
        nc.vector.tensor_copy(out=allones[:], in_=allones_f[:])

        qT = [persist.tile([P, S], f16, name=f"qT{b}") for b in range(NH)]
        kT = [persist.tile([P, S], f16, name=f"kT{b}") for b in range(NH)]
        vT = [persist.tile([P, S], f16, name=f"vT{b}") for b in range(NH)]

        a2a_in = [dram.tile([W, P, QS], f16, name=f"a2a_in{b}") for b in range(NH)]
        a2a_out = [dram.tile([W, P, QS], f16, name=f"a2a_out{b}") for b in range(NH)]

        # ---------------- Phase A: projections ----------------
        psA = tc.alloc_tile_pool(name="psA", bufs=2, space="PSUM")
        wpool = tc.alloc_tile_pool(name="wpool", bufs=6)
        xpool = tc.alloc_tile_pool(name="xpool", bufs=1)
        HALF = S // 2
        dests = {"q": qT, "k": kT, "v": vT}
        dma_engines = [nc.sync, nc.scalar]

        def proj(kind, b, xts, col0):
            wt = wtiles[(kind, b)]
            dst = dests[kind][b]
            for c in range(2):
                ps = psA.tile([P, 512], f32, name="psa", tag="psa")
                for t in range(DT):
                    nc.tensor.matmul(ps[:], wt[:, t, :],
                                     xts[t][:, c * 512:(c + 1) * 512],
                                     start=(t == 0), stop=(t == DT - 1))
                nc.scalar.copy(dst[:, col0 + c * 512:col0 + (c + 1) * 512],
                               ps[:])

        with nc.named_scope("A"):
            wtiles = {}
            for kind in ("k", "v", "q"):
                for b in range(NH):
                    wt = wpool.tile([P, DT, HD], f16, name=f"w_{kind}{b}",
                                    tag="wt")
                    nc.gpsimd.dma_start(wt[:], wqkv[kind][b])
                    wtiles[(kind, b)] = wt
            for half in range(2):
                col0 = half * HALF
                xts = []
                for t in range(DT):
                    xtile = xpool.tile([P, HALF], f16, name=f"x{half}_{t}",
                                       tag=f"x{half}_{t}")
                    dma_engines[t % 2].dma_start(
                        xtile[:], xt[t * P:(t + 1) * P, col0:col0 + HALF])
                    xts.append(xtile)
                if half == 0:
                    for kind in ("k", "v"):
                        for b in range(NH):
                            proj(kind, b, xts, col0)
                    for b in range(NH):
                        proj("q", b, xts, col0)
                else:
                    # head-0 K,V first so attention can begin early
                    for b in range(NH):
                        proj("k", b, xts, col0)
                        proj("v", b, xts, col0)
                    for b in range(NH):
                        proj("q", b, xts, col0)

        # ---------------- Phase B: attention ----------------
        maskp = tc.alloc_tile_pool(name="maskp", bufs=20)
        vpool = tc.alloc_tile_pool(name="vpool", bufs=1)
        psS = tc.alloc_tile_pool(name="psS", bufs=2, space="PSUM")
        psPV = tc.alloc_tile_pool(name="psPV", bufs=1, space="PSUM")

        v_sb = [vpool.tile([P, KT, P], f16, name=f"v_sb{b}") for b in range(NH)]
        # v transpose: vT [dh, S] -> v_sb [kv, kv_tile, dh]
        for b in range(NH):
          with nc.named_scope("T"):
            for t in range(KT):
                pst = psS.tile([P, P], f16, name="pst", tag="sc")
                nc.tensor.transpose(pst[:], vT[b][:, t * P:(t + 1) * P],
                                    ident_h[:])
                nc.vector.tensor_copy(out=v_sb[b][:, t, :], in_=pst[:])

        for b in range(NH):
          with nc.named_scope(f"B{b}"):
            for blk in range(NB):
                base = blk * QB
                mts = []
                for t in range(KT):
                    mtile = maskp.tile([P, QB], f16, name=f"mt{t}", tag="mt")
                    dma_engines[t % 2].dma_start(
                        mtile[:],
                        emask_t[b, t * P:(t + 1) * P, base:base + QB])
                    mts.append(mtile)
                pv = psPV.tile([P, QB], f32, name="pv", tag="pv")
                sacc = smallp.tile([P, QB], f16, name="sacc", tag="sacc")
                PIPE = 2
                pq = {}
                for tt in range(KT + PIPE):
                    if tt < KT:
                        t = tt
                        ktile = kT[b][:, t * P:(t + 1) * P]
                        ps_s = psS.tile([P, QB], f32, name="ps_s", tag="sc")
                        nc.tensor.matmul(ps_s[:, 0:512], ktile,
                                         qT[b][:, base:base + 512],
                                         start=True, stop=True)
                        nc.tensor.matmul(ps_s[:, 512:QB], ktile,
                                         qT[b][:, base + 512:base + QB],
                                         start=True, stop=True)
                        pr = probsp.tile([P, QB], f16, name="pr", tag="probs")
                        nc.scalar.activation(pr[:], ps_s[:], Exp)
                        nc.vector.tensor_tensor(out=pr[:], in0=pr[:],
                                                in1=mts[t][:], op=MULT)
                        if t == 0:
                            nc.vector.tensor_copy(out=sacc[:], in_=pr[:])
                        else:
                            nc.vector.tensor_tensor(out=sacc[:], in0=sacc[:],
                                                    in1=pr[:], op=ADD)
                        pq[t] = pr
                    if tt >= PIPE:
                        t = tt - PIPE
                        pr = pq.pop(t)
                        vtile = v_sb[b][:, t, :]
                        nc.tensor.matmul(pv[:, 0:512], vtile, pr[:, 0:512],
                                         start=(t == 0), stop=(t == KT - 1))
                        nc.tensor.matmul(pv[:, 512:QB], vtile, pr[:, 512:QB],
                                         start=(t == 0), stop=(t == KT - 1))
                # epilogue: fused partition-reduce + broadcast, normalize
                ps_bc = psS.tile([P, QB], f32, name="ps_bc", tag="sc")
                nc.tensor.matmul(ps_bc[:, 0:512], allones[:],
                                 sacc[:, 0:512], start=True, stop=True)
                nc.tensor.matmul(ps_bc[:, 512:QB], allones[:],
                                 sacc[:, 512:QB], start=True, stop=True)
                bc_sb = smallp.tile([P, QB], f32, name="bc_sb", tag="bc_sb")
                nc.vector.reciprocal_approx_fast(out=bc_sb[:], in_=ps_bc[:])
                attn_sb = smallp.tile([P, QB], f16, name="attn_sb",
                                      tag="attn_sb")
                nc.vector.tensor_tensor(out=attn_sb[:], in0=pv[:],
                                        in1=bc_sb[:], op=MULT)
                for jj in range(QB // QS):
                    j = blk * (QB // QS) + jj
                    nc.gpsimd.dma_start(a2a_in[b][j, :, :],
                                        attn_sb[:, jj * QS:(jj + 1) * QS])
            nc.gpsimd.collective_compute(
                "AllToAll", mybir.AluOpType.bypass,
                replica_groups=[list(range(W))],
                ins=[a2a_in[b].opt()],
                outs=[a2a_out[b].opt()],
            )
        for p in [psPV, psS, vpool, maskp, xpool, wpool, psA]:
            p.release()

        # ---------------- Phase D: output projection ----------------
        psD = tc.alloc_tile_pool(name="psD", bufs=1, space="PSUM")
        dpool = tc.alloc_tile_pool(name="dpool", bufs=3)
        ps_y = [[psD.tile([P, 512], f32, name=f"ps_y{qb}_{dc}", tag=f"ps_y{qb}_{dc}")
                 for dc in range(4)] for qb in range(2)]
        first = True
        for b in range(NH):
            for i in range(W):
                at = dpool.tile([P, QS], f16, name="at", tag="at", bufs=5)
                nc.sync.dma_start(at[:], a2a_out[b][i, :, :])
                h = 2 * i + b
                wo_sb = dpool.tile([P, D], f16, name="wo_sb", tag="wo_sb", bufs=8)
                nc.scalar.dma_start(wo_sb[:], wo_t[h * P:(h + 1) * P, :])
                last = (b == NH - 1) and (i == W - 1)
                for qb in range(2):
                    for dc in range(4):
                        nc.tensor.matmul(ps_y[qb][dc][:],
                                         at[:, qb * P:(qb + 1) * P],
                                         wo_sb[:, dc * 512:(dc + 1) * 512],
                                         start=first, stop=last)
                first = False
        for qb in range(2):
            for dc in range(4):
                y_sb = dpool.tile([P, 512], f32, name="y_sb", tag="y_sb",
                                  bufs=3)
                nc.vector.tensor_copy(out=y_sb[:], in_=ps_y[qb][dc][:])
                nc.gpsimd.dma_start(
                    y[qb * P:(qb + 1) * P, dc * 512:(dc + 1) * 512], y_sb[:])
        dpool.release()
        psD.release()

        for p in [dram, smallp, probsp, consts, persist]:
            p.release()

    nc.compile()
    return nc


def _warr(w, scale=None):
    """[D, 256] -> [NH, P, DT, HD] host layout (contiguous per head)."""
    wt = w.T if scale is None else (w * scale).T          # [D, 256]
    a = wt.reshape(DT, P, NH, HD).transpose(2, 1, 0, 3)   # [NH, P, DT, HD]
    return np.ascontiguousarray(a).astype(np.float16)


def make_in_maps(x, mask, wq, wk, wv, wo):
    """x [1,S,D]; mask [1,16,S,S]; w* [D,D] (all f32) -> per-core dicts."""
    scale = np.float32(1.0 / math.sqrt(HD))
    xtv = np.ascontiguousarray(x[0].T).astype(np.float16)
    wo_tv = np.ascontiguousarray(wo.T).astype(np.float16)
    in_maps = []
    for c in range(W):
        rows = slice(NH * HD * c, NH * HD * (c + 1))
        m = mask[0, NH * c:NH * (c + 1)]
        em = np.exp(m.transpose(0, 2, 1))
        in_maps.append({
            "xt": xtv,
            "wq_t": _warr(wq[rows], scale),
            "wk_t": _warr(wk[rows]),
            "wv_t": _warr(wv[rows]),
            "emask_t": np.ascontiguousarray(em).astype(np.float16),
            "wo_t": wo_tv,
        })
    return in_maps


def assemble(results):
    return np.concatenate([results[c]["y"] for c in range(W)], axis=0)[None]


# ----------------------------------------------------------------------
# Harness entry point: kernel(**inputs) takes the FULL unsharded inputs
# as produced by setup_inputs() and returns the FULL [1, S, D] output.
# Inside: inputs are sharded head-wise (TP) across the 8 NeuronCores,
# the Bass kernel runs SPMD (with two AllToAll collectives), and the
# seq-sharded outputs are concatenated on the host.
# ----------------------------------------------------------------------
_NC_CACHE = []


def kernel(x, mask, start_pos, wq, wk, wv, wo):
    from concourse import bass_utils
    x = np.asarray(x, dtype=np.float32)
    mask = np.asarray(mask, dtype=np.float32)
    wq = np.asarray(wq, dtype=np.float32)
    wk = np.asarray(wk, dtype=np.float32)
    wv = np.asarray(wv, dtype=np.float32)
    wo = np.asarray(wo, dtype=np.float32)
    # start_pos == 0 prefill (as in the reference)
    if not _NC_CACHE:
        _NC_CACHE.append(build())
    nc = _NC_CACHE[0]
    in_maps = make_in_maps(x, mask, wq, wk, wv, wo)
    res = bass_utils.run_bass_kernel_spmd(nc, in_maps, core_ids=list(range(W)))
    return assemble(res.results).astype(np.float32)
